# revision 32
# baseline (speedup 1.0000x reference)
"""Trainium2 Bass kernel for the KAN-style layer (nn_KAN_12936441496127).

Strategy: data-parallel over batch across 8 NeuronCores (256 rows/core).

The per-element map y_[b,o,i] = G(x[b,i], wn[o,i]) + b5(wn)*x depends only on
the scalar pair (x, wn), so G is fit host-side (on a dense wn-grid, linearly
interpolated to the 65536 wn[o,i] values) as a K-term separable expansion

    G(x, wn) ~= beta_0(wn) + beta_1(wn)*x + beta_2(wn)*x^2
                + beta_3(wn)*log1p(0.7x) + beta_4(wn)*log1p(3x)
                + beta_5(wn)*exp(-1.5x)

which turns the 134M-element transcendental chain into 5 bf16 matmuls:
y[b,o] = sum_i A_k[i,o]*phi_k(x[b,i]) + off[o], then softplus.  On device:
relu + basis on ACT (one table set) and DVE, 20 matmuls, Exp/Ln epilogue.
"""
import types
import numpy as np
from contextlib import ExitStack

import concourse.bass as bass
from concourse import bacc
from concourse import hw_specs
import concourse.tile as tile
from concourse import mybir
from concourse.bass_utils import run_bass_kernel_spmd

f32 = mybir.dt.float32
AF = mybir.ActivationFunctionType
ALU = mybir.AluOpType

B, IN, OUT = 2048, 256, 256
NCORES = 8
BL = B // NCORES          # 256 batch rows per core
K = 5                     # matmul basis funcs (constant handled via bias)
SC = 512.0                # global table scale, removed in epilogue Exp scale
C_LOG = (1.5,)
C_EXP = 1.5
WCLIP_LO, WCLIP_HI = 5.5, 35.5
ACT_SET = "natural_log_exp_and_others"
MM_DT = mybir.dt.float16
MM_NP = np.float16

_CACHE = {}


def _single_set_act_loads(self):
    """Instance override of Bacc.insert_act_table_loads: expose only the
    combined ln+exp set to the load-placement pass so every activation in
    this kernel shares one ACT_TABLE_LOAD (the default greedy pass picks
    natural_log then exp_and_others then natural_log = 3 loads)."""
    import bass_rust as _bass_rust

    has_activation = any(
        isinstance(i, mybir.InstActivation)
        for b in self.main_func.blocks
        for i in b.instructions
    )
    if not has_activation:
        return
    tables = [
        (n, (s if n == ACT_SET else set()))
        for n, s in hw_specs.get_activation_tables(self.m.arch).items()
    ]
    _bass_rust.insert_act_table_loads(self, tables)


def _build_bass():
    nc = bacc.Bacc("TRN2", target_bir_lowering=False, debug=False)
    nc.insert_act_table_loads = types.MethodType(_single_set_act_loads, nc)
    xT = nc.dram_tensor("xT", [IN, BL], mybir.dt.float16,
                        kind="ExternalInput").ap()
    tabs_d = [nc.dram_tensor(f"t{k}", [IN, OUT], MM_DT,
                             kind="ExternalInput").ap() for k in range(K)]
    off_d = nc.dram_tensor("off", [128, 2], f32, kind="ExternalInput").ap()
    yT = nc.dram_tensor("yT", [OUT, BL], mybir.dt.float16,
                        kind="ExternalOutput").ap()

    with tile.TileContext(nc) as tc, ExitStack() as ctx:
        pool = ctx.enter_context(tc.tile_pool(name="p", bufs=1))
        psum = ctx.enter_context(tc.tile_pool(name="ps", bufs=1, space="PSUM"))

        # warm the PE p-state during the DMA window: ~18 dummy matmuls keep
        # the PE continuously busy >3us so the clock ramps 0.65 -> 2.4 GHz
        # before the real matmuls arrive
        warm = pool.tile([128, BL], MM_DT, tag="warm")
        nc.vector.memset(warm[:], 0.0)
        ps_w = psum.tile([128, BL], f32, tag="ps_w", name="ps_w")
        for _ in range(12):
            nc.tensor.matmul(ps_w[:], warm[:, :128], warm[:],
                             start=True, stop=True)

        # ---- loads ----
        # x arrives already relu'd + fp16 from the host fold (the host owns
        # all elementwise input prep, the device owns the O(B*OUT*IN) work).
        # Four 32KB pieces spread over all three DMA queues minimize the
        # desc+transfer latency before compute can start.
        xpk = pool.tile([128, 2 * BL], MM_DT, tag="xpk")
        H = BL // 2

        def xdma(eng, ci, bh):
            eng.dma_start(
                xpk[:, ci * BL + bh * H: ci * BL + (bh + 1) * H],
                xT[ci * 128:(ci + 1) * 128, bh * H:(bh + 1) * H])

        xdma(nc.scalar, 0, 0)
        xdma(nc.sync, 1, 0)
        xdma(nc.gpsimd, 0, 1)
        xdma(nc.scalar, 1, 1)
        # table chunks spread over the 3 DMA queues by need-time (lin first,
        # exp last -- its matmuls run last anyway)
        tabs = [pool.tile([128, 2 * OUT], MM_DT, tag=f"t{k}", name=f"t{k}")
                for k in range(K)]

        def tdma(eng, k, ci):
            eng.dma_start(tabs[k][:, ci * OUT:(ci + 1) * OUT],
                          tabs_d[k][ci * 128:(ci + 1) * 128, :])

        tdma(nc.scalar, 0, 0)
        tdma(nc.sync, 0, 1)
        tdma(nc.gpsimd, 1, 0)
        tdma(nc.gpsimd, 1, 1)
        tdma(nc.sync, 2, 0)
        tdma(nc.sync, 2, 1)
        tdma(nc.sync, 3, 0)
        tdma(nc.gpsimd, 3, 1)
        tdma(nc.sync, 4, 0)
        tdma(nc.gpsimd, 4, 1)
        offt = pool.tile([128, 2], f32, tag="offt")
        nc.gpsimd.dma_start(offt[:], off_d[:])

        # ---- basis ----
        # x itself is the linear basis; sq/cube on DVE (fp16 2x mode);
        # log/exp on ACT.  All ACT funcs live in the single loaded set.
        basis = [xpk]
        b_sq = pool.tile([128, 2 * BL], MM_DT, tag="b_sq")
        nc.vector.tensor_tensor(b_sq[:], xpk[:], xpk[:], op=ALU.mult)
        basis.append(b_sq)
        b_cu = pool.tile([128, 2 * BL], MM_DT, tag="b_cu")
        nc.vector.tensor_tensor(b_cu[:], b_sq[:], xpk[:], op=ALU.mult)
        basis.append(b_cu)
        for c in C_LOG:
            bl_ = pool.tile([128, 2 * BL], MM_DT, tag=f"b_l{c}")
            nc.scalar.activation(bl_[:], xpk[:], AF.Ln, bias=1.0,
                                 scale=float(c))
            basis.append(bl_)
        b_e = pool.tile([128, 2 * BL], MM_DT, tag="b_e")
        nc.scalar.activation(b_e[:], xpk[:], AF.Exp, scale=-float(C_EXP))
        basis.append(b_e)

        # ---- matmuls: psum[co] += A_k[ci,:,co].T @ phi_k[ci] ----
        # co-major: finish the co=0 accumulation first so its softplus
        # epilogue + store overlap the co=1 matmuls.  A dep-free warmup
        # matmul between groups keeps the PE busy across the semaphore
        # waits so the clock never drops back from its ramped state.
        ps = [psum.tile([128, BL], f32, tag=f"ps{co}", name=f"ps{co}")
              for co in range(2)]
        for co in range(2):
            n = 0
            for k in range(K):
                if co == 0:
                    nc.tensor.matmul(ps_w[:], warm[:, :128], warm[:],
                                     start=True, stop=True)
                for ci in range(2):
                    nc.tensor.matmul(
                        ps[co][:],
                        tabs[k][:, ci * OUT + co * 128: ci * OUT + (co + 1) * 128],
                        basis[k][:, ci * BL:(ci + 1) * BL],
                        start=(n == 0), stop=(n == 2 * K - 1),
                    )
                    n += 1

        # ---- softplus epilogue: ln(1 + exp(psum/SC + off)) ----
        # per-co biased Exp (Exp0 overlaps the co=1 matmuls), one wide Ln,
        # then both output descs in parallel on the sync + scalar queues
        es = pool.tile([128, 2 * BL], f32, tag="es")
        for co in range(2):
            nc.scalar.activation(es[:, co * BL:(co + 1) * BL], ps[co][:],
                                 AF.Exp, bias=offt[:, co:co + 1],
                                 scale=1.0 / SC)
        yo = pool.tile([128, 2 * BL], mybir.dt.float16, tag="yo")
        nc.scalar.activation(yo[:], es[:], AF.Ln, bias=1.0)
        nc.sync.dma_start(yT[:128, :], yo[:, :BL])
        nc.scalar.dma_start(yT[128:, :], yo[:, BL:])
    nc.compile()
    return nc


def _eval_splines(wn, breaks, coefs):
    """wn [...], breaks [5,17], coefs [5,16,4] -> list of 5 arrays."""
    out = []
    for s in range(breaks.shape[0]):
        br = np.asarray(breaks[s], np.float64)
        cf = np.asarray(coefs[s], np.float64)
        wc = np.clip(wn, br[0], br[-1] - 1e-6)
        idx = np.clip(np.searchsorted(br, wc, side="right") - 1, 0,
                      cf.shape[0] - 1)
        a = cf[idx]
        t = wc - br[idx]
        out.append(((a[..., 0] * t + a[..., 1]) * t + a[..., 2]) * t
                   + a[..., 3])
    return out


def _fold_params(w, raw_gamma, breaks, coefs, mu, sigma, xmax):
    w = np.asarray(w, np.float64)
    mu = float(mu)
    sigma = float(sigma)
    wn = (np.clip(w, WCLIP_LO, WCLIP_HI) - mu) / sigma          # [OUT, IN]
    wn_lo = (WCLIP_LO - mu) / sigma
    wn_hi = (WCLIP_HI - mu) / sigma

    # ---- fit beta(wn) on a dense wn grid ----
    M = 4097
    wn_g = np.linspace(wn_lo, wn_hi, M)
    b1g, b2g, b3g, b4g, b5g = _eval_splines(wn_g, breaks, coefs)  # [M] each

    GRID = 512
    kk = np.arange(GRID)
    sg = np.cos(np.pi * (kk + 0.5) / GRID)
    xg = (sg + 1.0) * (xmax / 2.0)                               # (0, xmax)
    wts = np.sqrt(np.exp(-xg * xg / 2.0) + 0.05)

    cols = [np.ones_like(xg), xg, xg * xg, xg ** 3]
    cols += [np.log1p(c * xg) for c in C_LOG]
    cols += [np.exp(-C_EXP * xg)]
    Phi = np.stack(cols, axis=1)                                 # [GRID, K+1]
    phi0 = np.array([1.0, 0.0, 0.0, 0.0] + [0.0] * len(C_LOG) + [1.0])

    u = b3g[:, None] * xg[None, :]                               # [M, GRID]
    base = np.expm1(u)
    Gv = b1g[:, None] * np.log1p(
        b2g[:, None] * np.log1p(base ** b4g[:, None]))           # [M, GRID]

    Wb = np.vstack([Phi * wts[:, None], phi0[None, :] * 1e4])
    Wg = np.hstack([Gv * wts[None, :], np.zeros((M, 1))])
    beta_g, *_ = np.linalg.lstsq(Wb, Wg.T, rcond=None)           # [K+1, M]

    # ---- interpolate at the actual wn[o,i] ----
    pos = (wn - wn_lo) / (wn_hi - wn_lo) * (M - 1)
    i0 = np.clip(np.floor(pos).astype(np.int64), 0, M - 2)
    fr = pos - i0
    beta = (beta_g[:, i0] * (1.0 - fr) + beta_g[:, i0 + 1] * fr)  # [K+1,OUT,IN]
    b5 = b5g[i0] * (1.0 - fr) + b5g[i0 + 1] * fr                  # [OUT, IN]
    beta[1] += b5                                                # linear term

    gam = np.logaddexp(np.asarray(raw_gamma, np.float64), 0.0) / OUT
    A = beta[1:] * gam[None, :, :] * SC                          # [K, OUT, IN]
    off = (beta[0] * gam).sum(axis=1)                            # [OUT]

    out = {f"t{k}": np.ascontiguousarray(A[k].T).astype(MM_NP)
           for k in range(K)}
    out["off"] = np.ascontiguousarray(
        off.astype(np.float32).reshape(2, 128).T)                # [128, 2]
    return out


def _run(nc, x, tabs, trace=False):
    xr = np.maximum(np.asarray(x, np.float32), 0.0)
    xT = np.ascontiguousarray(xr.astype(np.float16).T)           # [IN, B]
    in_maps = []
    for c in range(NCORES):
        m = {"xT": np.ascontiguousarray(xT[:, c * BL:(c + 1) * BL])}
        m.update(tabs)
        in_maps.append(m)
    res = run_bass_kernel_spmd(nc, in_maps, list(range(NCORES)), trace=trace)
    yT = np.concatenate([res.results[c]["yT"] for c in range(NCORES)], axis=1)
    return np.ascontiguousarray(yT.T.astype(np.float32)), res


def kernel(x, w, raw_gamma, breaks, coefs, mu, sigma):
    if "nc" not in _CACHE:
        _CACHE["nc"] = _build_bass()
    x = np.asarray(x, np.float32)
    xmax = max(4.0, float(x.max()) * 1.000001)
    tabs = _fold_params(w, raw_gamma, breaks, coefs, mu, sigma, xmax)
    y, _ = _run(_CACHE["nc"], x, tabs)
    return y


# revision 35
# speedup vs baseline: 1.1339x; 1.1339x over previous
"""Trainium2 Bass kernel for the KAN-style layer (nn_KAN_12936441496127).

Strategy: data-parallel over batch across 8 NeuronCores (256 rows/core).

The per-element map y_[b,o,i] = G(x[b,i], wn[o,i]) + b5(wn)*x depends only on
the scalar pair (x, wn), so G is fit host-side (on a dense wn-grid, linearly
interpolated to the 65536 wn[o,i] values) as a K-term separable expansion

    G(x, wn) ~= beta_0(wn) + beta_1(wn)*x + beta_2(wn)*x^2
                + beta_3(wn)*log1p(0.7x) + beta_4(wn)*log1p(3x)
                + beta_5(wn)*exp(-1.5x)

which turns the 134M-element transcendental chain into 5 bf16 matmuls:
y[b,o] = sum_i A_k[i,o]*phi_k(x[b,i]) + off[o], then softplus.  On device:
relu + basis on ACT (one table set) and DVE, 20 matmuls, Exp/Ln epilogue.
"""
import types
import numpy as np
from contextlib import ExitStack

import concourse.bass as bass
from concourse import bacc
from concourse import hw_specs
import concourse.tile as tile
from concourse import mybir
from concourse.bass_utils import run_bass_kernel_spmd

f32 = mybir.dt.float32
AF = mybir.ActivationFunctionType
ALU = mybir.AluOpType

B, IN, OUT = 2048, 256, 256
NCORES = 8
BL = B // NCORES          # 256 batch rows per core
K = 5                     # matmul basis funcs (constant handled via bias)
SC = 512.0                # global table scale, removed in epilogue Exp scale
C_LOG = (1.5,)
C_EXP = 1.5
WCLIP_LO, WCLIP_HI = 5.5, 35.5
ACT_SET = "natural_log_exp_and_others"
MM_DT = mybir.dt.float16
MM_NP = np.float16

_CACHE = {}


def _single_set_act_loads(self):
    """Instance override of Bacc.insert_act_table_loads: expose only the
    combined ln+exp set to the load-placement pass so every activation in
    this kernel shares one ACT_TABLE_LOAD (the default greedy pass picks
    natural_log then exp_and_others then natural_log = 3 loads)."""
    import bass_rust as _bass_rust

    has_activation = any(
        isinstance(i, mybir.InstActivation)
        for b in self.main_func.blocks
        for i in b.instructions
    )
    if not has_activation:
        return
    tables = [
        (n, (s if n == ACT_SET else set()))
        for n, s in hw_specs.get_activation_tables(self.m.arch).items()
    ]
    _bass_rust.insert_act_table_loads(self, tables)


def _build_bass():
    nc = bacc.Bacc("TRN2", target_bir_lowering=False, debug=False)
    nc.insert_act_table_loads = types.MethodType(_single_set_act_loads, nc)
    xT = nc.dram_tensor("xT", [IN, BL], mybir.dt.float16,
                        kind="ExternalInput").ap()
    tabs_d = [nc.dram_tensor(f"t{k}", [IN, OUT], MM_DT,
                             kind="ExternalInput").ap() for k in range(K)]
    off_d = nc.dram_tensor("off", [128, 2], f32, kind="ExternalInput").ap()
    yT = nc.dram_tensor("yT", [OUT, BL], mybir.dt.float16,
                        kind="ExternalOutput").ap()

    with tile.TileContext(nc) as tc, ExitStack() as ctx:
        pool = ctx.enter_context(tc.tile_pool(name="p", bufs=1))
        psum = ctx.enter_context(tc.tile_pool(name="ps", bufs=1, space="PSUM"))

        # warm the PE p-state during the DMA window: ~18 dummy matmuls keep
        # the PE continuously busy >3us so the clock ramps 0.65 -> 2.4 GHz
        # before the real matmuls arrive
        warm = pool.tile([128, BL], MM_DT, tag="warm")
        nc.vector.memset(warm[:], 0.0)
        ps_w = psum.tile([128, BL], f32, tag="ps_w", name="ps_w")
        for _ in range(14):
            nc.tensor.matmul(ps_w[:], warm[:, :128], warm[:],
                             start=True, stop=True)

        # ---- loads ----
        # x arrives already relu'd + fp16 from the host fold (the host owns
        # all elementwise input prep, the device owns the O(B*OUT*IN) work).
        # Four 32KB pieces spread over all three DMA queues minimize the
        # desc+transfer latency before compute can start.
        xpk = pool.tile([128, 2 * BL], MM_DT, tag="xpk")
        H = BL // 2

        def xdma(eng, ci, bh):
            eng.dma_start(
                xpk[:, ci * BL + bh * H: ci * BL + (bh + 1) * H],
                xT[ci * 128:(ci + 1) * 128, bh * H:(bh + 1) * H])

        xdma(nc.scalar, 0, 0)
        xdma(nc.sync, 1, 0)
        xdma(nc.gpsimd, 0, 1)
        xdma(nc.scalar, 1, 1)
        # table chunks spread over the 3 DMA queues by need-time (lin first,
        # exp last -- its matmuls run last anyway)
        tabs = [pool.tile([128, 2 * OUT], MM_DT, tag=f"t{k}", name=f"t{k}")
                for k in range(K)]

        def tdma(eng, k, ci):
            eng.dma_start(tabs[k][:, ci * OUT:(ci + 1) * OUT],
                          tabs_d[k][ci * 128:(ci + 1) * 128, :])

        tdma(nc.gpsimd, 0, 0)
        tdma(nc.sync, 0, 1)
        tdma(nc.gpsimd, 1, 0)
        tdma(nc.sync, 1, 1)
        tdma(nc.gpsimd, 2, 0)
        tdma(nc.sync, 2, 1)
        tdma(nc.sync, 3, 0)
        tdma(nc.scalar, 3, 1)
        tdma(nc.sync, 4, 0)
        tdma(nc.gpsimd, 4, 1)
        offt = pool.tile([128, 2], f32, tag="offt")
        nc.gpsimd.dma_start(offt[:], off_d[:])

        # ---- basis ----
        # x itself is the linear basis; sq/cube on DVE (fp16 2x mode);
        # log/exp on ACT.  All ACT funcs live in the single loaded set.
        basis = [xpk]
        b_sq = pool.tile([128, 2 * BL], MM_DT, tag="b_sq")
        nc.vector.tensor_tensor(b_sq[:], xpk[:], xpk[:], op=ALU.mult)
        basis.append(b_sq)
        b_cu = pool.tile([128, 2 * BL], MM_DT, tag="b_cu")
        nc.vector.tensor_tensor(b_cu[:], b_sq[:], xpk[:], op=ALU.mult)
        basis.append(b_cu)
        for c in C_LOG:
            bl_ = pool.tile([128, 2 * BL], MM_DT, tag=f"b_l{c}")
            nc.scalar.activation(bl_[:], xpk[:], AF.Ln, bias=1.0,
                                 scale=float(c))
            basis.append(bl_)
        b_e = pool.tile([128, 2 * BL], MM_DT, tag="b_e")
        nc.scalar.activation(b_e[:], xpk[:], AF.Exp, scale=-float(C_EXP))
        basis.append(b_e)

        # ---- matmuls: psum[co] += A_k[ci,:,co].T @ phi_k[ci] ----
        # co-major: finish the co=0 accumulation first so its softplus
        # epilogue + store overlap the co=1 matmuls.  A dep-free warmup
        # matmul between groups keeps the PE busy across the semaphore
        # waits so the clock never drops back from its ramped state.
        ps = [psum.tile([128, BL], f32, tag=f"ps{co}", name=f"ps{co}")
              for co in range(2)]
        for co in range(2):
            n = 0
            for k in range(K):
                if co == 0:
                    nc.tensor.matmul(ps_w[:], warm[:, :128], warm[:],
                                     start=True, stop=True)
                for ci in range(2):
                    nc.tensor.matmul(
                        ps[co][:],
                        tabs[k][:, ci * OUT + co * 128: ci * OUT + (co + 1) * 128],
                        basis[k][:, ci * BL:(ci + 1) * BL],
                        start=(n == 0), stop=(n == 2 * K - 1),
                    )
                    n += 1

        # ---- softplus epilogue: ln(1 + exp(psum/SC + off)) ----
        # pipelined per co (Exp0/Ln0/store overlap the co=1 matmuls);
        # out0 on the sync queue, out1 from scalar so the descs go out in
        # parallel
        oeng = [nc.sync, nc.scalar]
        for co in range(2):
            e = pool.tile([128, BL], f32, tag=f"e{co}", name=f"e{co}")
            nc.scalar.activation(e[:], ps[co][:], AF.Exp,
                                 bias=offt[:, co:co + 1], scale=1.0 / SC)
            yo = pool.tile([128, BL], mybir.dt.float16, tag=f"yo{co}",
                           name=f"yo{co}")
            nc.scalar.activation(yo[:], e[:], AF.Ln, bias=1.0)
            oeng[co].dma_start(yT[co * 128:(co + 1) * 128, :], yo[:])
    nc.compile()
    return nc


def _eval_splines(wn, breaks, coefs):
    """wn [...], breaks [5,17], coefs [5,16,4] -> list of 5 arrays."""
    out = []
    for s in range(breaks.shape[0]):
        br = np.asarray(breaks[s], np.float64)
        cf = np.asarray(coefs[s], np.float64)
        wc = np.clip(wn, br[0], br[-1] - 1e-6)
        idx = np.clip(np.searchsorted(br, wc, side="right") - 1, 0,
                      cf.shape[0] - 1)
        a = cf[idx]
        t = wc - br[idx]
        out.append(((a[..., 0] * t + a[..., 1]) * t + a[..., 2]) * t
                   + a[..., 3])
    return out


def _fold_params(w, raw_gamma, breaks, coefs, mu, sigma, xmax):
    w = np.asarray(w, np.float64)
    mu = float(mu)
    sigma = float(sigma)
    wn = (np.clip(w, WCLIP_LO, WCLIP_HI) - mu) / sigma          # [OUT, IN]
    wn_lo = (WCLIP_LO - mu) / sigma
    wn_hi = (WCLIP_HI - mu) / sigma

    # ---- fit beta(wn) on a dense wn grid ----
    M = 4097
    wn_g = np.linspace(wn_lo, wn_hi, M)
    b1g, b2g, b3g, b4g, b5g = _eval_splines(wn_g, breaks, coefs)  # [M] each

    GRID = 512
    kk = np.arange(GRID)
    sg = np.cos(np.pi * (kk + 0.5) / GRID)
    xg = (sg + 1.0) * (xmax / 2.0)                               # (0, xmax)
    wts = np.sqrt(np.exp(-xg * xg / 2.0) + 0.05)

    cols = [np.ones_like(xg), xg, xg * xg, xg ** 3]
    cols += [np.log1p(c * xg) for c in C_LOG]
    cols += [np.exp(-C_EXP * xg)]
    Phi = np.stack(cols, axis=1)                                 # [GRID, K+1]
    phi0 = np.array([1.0, 0.0, 0.0, 0.0] + [0.0] * len(C_LOG) + [1.0])

    u = b3g[:, None] * xg[None, :]                               # [M, GRID]
    base = np.expm1(u)
    Gv = b1g[:, None] * np.log1p(
        b2g[:, None] * np.log1p(base ** b4g[:, None]))           # [M, GRID]

    Wb = np.vstack([Phi * wts[:, None], phi0[None, :] * 1e4])
    Wg = np.hstack([Gv * wts[None, :], np.zeros((M, 1))])
    beta_g, *_ = np.linalg.lstsq(Wb, Wg.T, rcond=None)           # [K+1, M]

    # ---- interpolate at the actual wn[o,i] ----
    pos = (wn - wn_lo) / (wn_hi - wn_lo) * (M - 1)
    i0 = np.clip(np.floor(pos).astype(np.int64), 0, M - 2)
    fr = pos - i0
    beta = (beta_g[:, i0] * (1.0 - fr) + beta_g[:, i0 + 1] * fr)  # [K+1,OUT,IN]
    b5 = b5g[i0] * (1.0 - fr) + b5g[i0 + 1] * fr                  # [OUT, IN]
    beta[1] += b5                                                # linear term

    gam = np.logaddexp(np.asarray(raw_gamma, np.float64), 0.0) / OUT
    A = beta[1:] * gam[None, :, :] * SC                          # [K, OUT, IN]
    off = (beta[0] * gam).sum(axis=1)                            # [OUT]

    out = {f"t{k}": np.ascontiguousarray(A[k].T).astype(MM_NP)
           for k in range(K)}
    out["off"] = np.ascontiguousarray(
        off.astype(np.float32).reshape(2, 128).T)                # [128, 2]
    return out


def _run(nc, x, tabs, trace=False):
    xr = np.maximum(np.asarray(x, np.float32), 0.0)
    xT = np.ascontiguousarray(xr.astype(np.float16).T)           # [IN, B]
    in_maps = []
    for c in range(NCORES):
        m = {"xT": np.ascontiguousarray(xT[:, c * BL:(c + 1) * BL])}
        m.update(tabs)
        in_maps.append(m)
    res = run_bass_kernel_spmd(nc, in_maps, list(range(NCORES)), trace=trace)
    yT = np.concatenate([res.results[c]["yT"] for c in range(NCORES)], axis=1)
    return np.ascontiguousarray(yT.T.astype(np.float32)), res


def kernel(x, w, raw_gamma, breaks, coefs, mu, sigma):
    if "nc" not in _CACHE:
        _CACHE["nc"] = _build_bass()
    x = np.asarray(x, np.float32)
    xmax = max(4.0, float(x.max()) * 1.000001)
    tabs = _fold_params(w, raw_gamma, breaks, coefs, mu, sigma, xmax)
    y, _ = _run(_CACHE["nc"], x, tabs)
    return y
